# revision 19
# baseline (speedup 1.0000x reference)
"""Multi-head attention (B=2, S=2048, D=1024, H=16) on 8 Trainium2 NeuronCores.

Sharding: core c -> (batch b = c//4, head-group g = c%4).  Each core computes
Q/K/V projections for its 4 heads (256 features), causal attention for those
heads over the full sequence, and a partial O-projection (its 256 attn
features x full Wo.T slice).  The host sums the 4 partial outputs per batch
and folds in the biases that commute with the reduction (bo, bv @ Wo.T).

Device-side layout (per core, all matmul operands bf16, accumulation f32):
  Q^T, K^T  [feat, tok]   (feature-on-partition; per-partition bias on DVE)
  V         [tok, feat+1] (augmented with a ones column -> PV matmul also
                           accumulates the softmax denominator)
  scores^T  [k, q] tiles  -> exp on ScalarE with fused 1/sqrt(dk) scale; no
                           max-subtraction (scores are O(5) for this data,
                           exp is exact to 2 ULP and f32 can't overflow).
                           Two k-tiles share one [128,2,512] PSUM tile so a
                           single ACTIVATE covers both.
  masking   multiplicative bf16 tiles after exp; partially-masked tiles also
            carry a start column c0 so QK/exp/PV skip the dead q-range
  attnU^T + denom = V_aug^T @ P^T accumulated over k tiles in PSUM
  normalize: DVE reciprocal_approx_fast + GpSimd partition-broadcast + DVE mul
  O-proj    attn^T tiles stationary, Wo^T slice streaming -> partial out
            bf16, drained per 128-token tile and DMA'd out immediately

Software pipeline: the kernel runs `proj(0); for qb: attention(qb),
proj(qb+1), oproj(qb)`.  proj(qb+1) sits between attention(qb) and
oproj(qb) in program order so the Tile scheduler fills TensorE stalls
(waiting on ScalarE exp) with projection matmuls, and the shared psacc
PSUM pool's rotation never chains a projection group behind an
attention-dependent O-projection group.

All inputs are host-packed into SBUF layout (partition-major, contiguous
per partition) so each input DMA is 128 fat descriptors: descriptor
generation cost ~1.3us/transfer instead of ~3us, and the per-query-block
K/Q/V chunks arrive in consumption order on one FIFO queue.
"""

import hashlib
from contextlib import ExitStack

import ml_dtypes
import numpy as np

import concourse.bass as bass
import concourse.tile as tile
from concourse import bacc, mybir
from concourse.bass_utils import run_bass_kernel_spmd

B, S, D, H = 2, 2048, 1024, 16
DK = D // H                  # 64 head dim
NCORE = 8
GROUPS = NCORE // B          # 4 head-groups per batch
HPC = H // GROUPS            # 4 heads per core
FPC = HPC * DK               # 256 features per core
FT = FPC // 128              # 2 feature tiles per core
DT = D // 128                # 8 d_in tiles
TT = S // 128                # 16 token tiles (k tiles)
QB = 512                     # query block (free-dim) size in attention
NQB = S // QB                # 4 query blocks
NCH = 512                    # psum free-dim chunk for projections
BF = mybir.dt.bfloat16
F32 = mybir.dt.float32
BFNP = ml_dtypes.bfloat16

# module-level knobs for test.py
PROFILE = False
TRACE_CORES = None
LAST_RESULT = None

_program_cache: dict = {}


def _classify_mask(mask2d: np.ndarray):
    """Classify (S, S) keep-mask into per-(qblock, ktile) modes.

    Returns (plan, patterns): plan[qb] is a list of (kt, mask_id|None, c0)
    for tiles that are at least partially kept, where c0 is the first
    q-column (within the block) with any kept key; patterns is a list of
    [128, QB] bf16 multiplicative mask tiles (k on partitions, q free).
    """
    keep = np.asarray(mask2d) != 0
    patterns = []
    pattern_ids = {}
    plan = []
    for qb in range(NQB):
        row = []
        for kt in range(TT):
            blk = keep[qb * QB:(qb + 1) * QB, kt * 128:(kt + 1) * 128].T
            if not blk.any():
                continue
            if blk.all():
                row.append((kt, None, 0, 0))
                continue
            # c0: first column with any kept key (QK/exp/PV start here).
            # c1: first column from which every column is all-keep; only
            # [c0, c1) needs the multiplicative mask.
            anyk = blk.any(axis=0)
            allk = blk.all(axis=0)
            c0 = int(np.flatnonzero(anyk)[0])
            notall = np.flatnonzero(~allk)
            c1 = int(notall[-1]) + 1 if notall.size else c0
            pat = blk[:, c0:c1]
            key = pat.tobytes()
            mid = pattern_ids.get(key)
            if mid is None:
                mid = len(patterns)
                pattern_ids[key] = mid
                patterns.append(pat.astype(BFNP))
            row.append((kt, mid, c0, c1))
        plan.append(row)
    return plan, patterns


def build_program(plan, npat, pw, combos):
    nc = bacc.Bacc("TRN2", target_bir_lowering=False, debug=False,
                   num_devices=NCORE)
    # host-packed inputs: partition-major, contiguous per partition
    xkq = nc.dram_tensor("xkq", (128, NQB, 2, DT, 512), BF,
                         kind="ExternalInput").ap()
    xv4 = nc.dram_tensor("xv4", (128, NQB, 4, DT, 128), BF,
                         kind="ExternalInput").ap()
    wall = nc.dram_tensor("wall", (128, 3, DT, FPC), BF,
                          kind="ExternalInput").ap()
    wo = nc.dram_tensor("wo", (128, FT, D), BF, kind="ExternalInput").ap()
    bqk = nc.dram_tensor("bqk", (128, 2, FT), F32, kind="ExternalInput").ap()
    masks = None
    if npat:
        masks = nc.dram_tensor("masks", (128, npat, pw), BF,
                               kind="ExternalInput").ap()
    out = nc.dram_tensor("out", (S, D), BF, kind="ExternalOutput").ap()
    out_r = out.rearrange("(t p) d -> t p d", p=128)

    with tile.TileContext(nc) as tc, ExitStack() as ctx:
        singles = ctx.enter_context(tc.tile_pool(name="singles", bufs=1))
        xpool = ctx.enter_context(tc.tile_pool(name="xpool", bufs=2))
        ppool = ctx.enter_context(tc.tile_pool(name="ppool", bufs=6))
        npool = ctx.enter_context(tc.tile_pool(name="npool", bufs=4))
        opool = ctx.enter_context(tc.tile_pool(name="opool", bufs=4))
        psacc = ctx.enter_context(tc.tile_pool(name="psacc", bufs=2, space="PSUM"))
        psS = ctx.enter_context(tc.tile_pool(name="psS", bufs=2, space="PSUM"))
        psPV = ctx.enter_context(tc.tile_pool(name="psPV", bufs=2, space="PSUM"))

        # ---- SBUF residents ----
        w_sb = singles.tile([128, 3, DT, FPC], BF)     # wk | wq | wv
        wo_sb = singles.tile([128, FT, D], BF)
        bias_sb = singles.tile([128, 2, FT], F32)
        mask_sb = None
        if npat:
            mask_sb = singles.tile([128, npat, pw], BF, name="mask_sb")
        q_sb = singles.tile([128, FT, S], BF)
        k_sb = singles.tile([128, FT, S], BF)
        attn_sb = singles.tile([128, FT, S], BF)
        v_sb = singles.tile([128, TT, HPC, DK + 1], BF)
        # per-chunk k|q and v input tiles; bufs=2 so the dma_start for
        # chunk c waits for chunk c-2's last projection read -> later
        # chunks don't steal DMA bandwidth from earlier-needed data
        x_ch = [xpool.tile([128, 2, DT, 512], BF, tag="xkq", name=f"x_ch{c}")
                for c in range(NQB)]
        xv_ch = [xpool.tile([128, 4, DT, 128], BF, tag="xv", name=f"xv_ch{c}")
                 for c in range(NQB)]

        # ---- DMA issue order == consumption order, one FIFO queue.
        # V weights + first token tiles lead so TensorE can start V
        # projections ~10us in, while the K/Q chunk data still streams. ----
        nc.sync.dma_start(bias_sb, bqk)
        nc.sync.dma_start(w_sb[:, 2], wall[:, 2])
        nc.sync.dma_start(xv_ch[0][:, 0:2], xv4[:, 0, 0:2])
        nc.sync.dma_start(w_sb[:, 0], wall[:, 0])
        nc.sync.dma_start(x_ch[0][:, 0], xkq[:, 0, 0])
        nc.sync.dma_start(xv_ch[0][:, 2:4], xv4[:, 0, 2:4])
        nc.sync.dma_start(w_sb[:, 1], wall[:, 1])
        nc.sync.dma_start(x_ch[0][:, 1], xkq[:, 0, 1])
        if npat:
            nc.sync.dma_start(mask_sb, masks)
        # dummy readback of chunk 0: stalls the sync DGE queue until chunk 0
        # has landed, so chunk 1's transfer can't steal DMA bandwidth from
        # the data the first projections are waiting on.  The target rows of
        # `out` are overwritten by the real output DMAs later on this queue.
        nc.sync.dma_start(out[0:1, 0:64], x_ch[0][0:1, 1, DT - 1, 448:])
        nc.sync.dma_start(x_ch[1], xkq[:, 1])
        nc.sync.dma_start(xv_ch[1], xv4[:, 1])
        nc.sync.dma_start(wo_sb, wo)
        for c in range(2, NQB):
            nc.sync.dma_start(x_ch[c], xkq[:, c])
            nc.sync.dma_start(xv_ch[c], xv4[:, c])

        for tt in range(TT):
            nc.vector.memset(v_sb[:, tt, :, DK:DK + 1], 1.0)

        def proj_kq(which, nch):
            # K^T / Q^T projection chunk ([feat, tok] layout, bias on DVE)
            xi, wi, y_sb, bi = ((0, 0, k_sb, 1) if which == "k"
                                else (1, 1, q_sb, 0))
            for ft in range(FT):
                ps = psacc.tile([128, NCH], F32, tag="acc")
                for dt in range(DT):
                    nc.tensor.matmul(
                        ps,
                        lhsT=w_sb[:, wi, dt, ft * 128:(ft + 1) * 128],
                        rhs=x_ch[nch][:, xi, dt, :],
                        start=(dt == 0), stop=(dt == DT - 1))
                nc.vector.tensor_scalar_add(
                    y_sb[:, ft, nch * NCH:(nch + 1) * NCH], ps,
                    bias_sb[:, bi, ft:ft + 1])

        def proj_v(tt):
            # V projection token tile (natural layout [tok, feat])
            c, r = divmod(tt, 4)
            ps = psacc.tile([128, FPC], F32, tag="acc")
            for dt in range(DT):
                nc.tensor.matmul(ps,
                                 lhsT=xv_ch[c][:, r, dt, :],
                                 rhs=w_sb[:, 2, dt, :],
                                 start=(dt == 0), stop=(dt == DT - 1))
            nc.vector.tensor_copy(v_sb[:, tt, :, 0:DK],
                                  ps.rearrange("p (h d) -> p h d", h=HPC))

        def proj(qb):
            # qb=0 runs V first: its data leads the DMA stream, so TensorE
            # has work while the K/Q chunks are still arriving
            if qb == 0:
                for tt in range(4):
                    proj_v(tt)
            proj_kq("k", qb)
            proj_kq("q", qb)
            if qb > 0:
                for tt in range(qb * 4, qb * 4 + 4):
                    proj_v(tt)

        inv_sqrt_dk = float(1.0 / np.sqrt(DK))

        def attention(qb):
            kts = plan[qb]
            for h in range(HPC):
                if not kts:
                    continue
                pr = (h % 2) * 64
                ft = h // 2
                pairs = [kts[i:i + 2] for i in range(0, len(kts), 2)]
                pv = psPV.tile([DK + 1, QB], F32)
                n = 0
                for pair in pairs:
                    pc0 = min(c0 for (_, _, c0, _) in pair)
                    s_ps = psS.tile([128, 2, QB], F32)
                    for j, (kt, _, c0, _) in enumerate(pair):
                        nc.tensor.matmul(
                            s_ps[:, j, c0:],
                            lhsT=k_sb[pr:pr + DK, ft, kt * 128:(kt + 1) * 128],
                            rhs=q_sb[pr:pr + DK, ft,
                                     qb * QB + c0:(qb + 1) * QB],
                            start=True, stop=True)
                    pT = ppool.tile([128, 2, QB], BF, tag="pt")
                    nc.scalar.activation(pT[:, 0:len(pair), pc0:],
                                         s_ps[:, 0:len(pair), pc0:],
                                         mybir.ActivationFunctionType.Exp,
                                         scale=inv_sqrt_dk)
                    combo = None
                    if len(pair) == 2:
                        (k0, m0, a0, b0), (k1, m1, a1, b1) = pair
                        if m0 is not None and m1 is not None:
                            combo = combos.get((m0, m1, b0 - a0, b1 - a1))
                    if combo is not None:
                        cmid, w = combo
                        assert mask_sb is not None
                        sl = pT[:, 0, a0:a0 + w]
                        both = bass.AP(
                            tensor=sl.tensor, offset=sl.offset,
                            ap=[sl.ap[0], [QB + (a1 - a0), 2], sl.ap[-1]])
                        nc.vector.tensor_mul(
                            both, both,
                            mask_sb[:, cmid, 0:2 * w].rearrange(
                                "p (j w) -> p j w", j=2))
                    else:
                        for j, (kt, mid, c0, c1) in enumerate(pair):
                            if mid is not None and c1 > c0:
                                assert mask_sb is not None
                                nc.vector.tensor_mul(
                                    pT[:, j, c0:c1], pT[:, j, c0:c1],
                                    mask_sb[:, mid, 0:c1 - c0])
                    for j, (kt, _, c0, _) in enumerate(pair):
                        nc.tensor.matmul(pv[:, c0:], lhsT=v_sb[:, kt, h, :],
                                         rhs=pT[:, j, c0:],
                                         start=(n == 0),
                                         stop=(n == len(kts) - 1))
                        n += 1
                # normalize: attn^T[d, q] = attnU^T[d, q] / denom[q]
                den = npool.tile([1, QB], F32, tag="den")
                nc.vector.tensor_copy(den, pv[DK:DK + 1, :])
                rec = npool.tile([1, QB], F32, tag="rec")
                nc.vector.reciprocal_approx_fast(rec, den)
                bc = npool.tile([64, QB], F32, tag="bc")
                nc.gpsimd.partition_broadcast(bc, rec)
                dst = attn_sb[pr:pr + DK, ft, qb * QB:(qb + 1) * QB]
                nc.vector.tensor_mul(dst, pv[0:DK, :], bc)

        def oproj(qb):
            # partial O-projection; bf16 staging per 128-token tile, DMA'd
            # out immediately so the final block's tail is short
            for qt in range(qb * QB // 128, (qb + 1) * QB // 128):
                ob = opool.tile([128, D], BF)
                for nch in range(D // NCH):
                    ps = psacc.tile([128, NCH], F32, tag="acc")
                    for hd in range(FT):
                        nc.tensor.matmul(
                            ps,
                            lhsT=attn_sb[:, hd, qt * 128:(qt + 1) * 128],
                            rhs=wo_sb[:, hd, nch * NCH:(nch + 1) * NCH],
                            start=(hd == 0), stop=(hd == FT - 1))
                    nc.vector.tensor_copy(ob[:, nch * NCH:(nch + 1) * NCH], ps)
                nc.sync.dma_start(out_r[qt], ob)

        # ---- software-pipelined main loop ----
        proj(0)
        for qb in range(NQB):
            attention(qb)
            if qb + 1 < NQB:
                proj(qb + 1)
            oproj(qb)

    nc.compile()
    return nc


def _get_program(mask2d: np.ndarray):
    key = hashlib.sha1(np.ascontiguousarray(mask2d).tobytes()).hexdigest()
    hit = _program_cache.get(key)
    if hit is not None:
        return hit
    plan, patterns = _classify_mask(mask2d)
    # combine the two masked tiles of a k-tile pair into one [128, 2, w]
    # pattern so a single strided DVE multiply covers both strips
    combos = {}
    for row in plan:
        for i in range(0, len(row) - 1, 2):
            (k0, m0, a0, b0), (k1, m1, a1, b1) = row[i], row[i + 1]
            if m0 is None or m1 is None:
                continue
            w = max(b0 - a0, b1 - a1)
            if a0 + w > QB or a1 + w > QB or w <= 0 or w > 128:
                continue
            key = (m0, m1, b0 - a0, b1 - a1)
            if key in combos:
                continue
            cp = np.ones((128, 2, w), BFNP)
            cp[:, 0, :b0 - a0] = patterns[m0]
            cp[:, 1, :b1 - a1] = patterns[m1]
            combos[key] = (len(patterns), w)
            patterns.append(cp.reshape(128, 2 * w))
    pw = max((p.shape[1] for p in patterns), default=0)
    nc = build_program(plan, len(patterns), pw, combos)
    if patterns:
        pat = np.zeros((len(patterns), 128, pw), BFNP)
        for i, p in enumerate(patterns):
            pat[i, :, :p.shape[1]] = p
        pat = np.ascontiguousarray(pat.transpose(1, 0, 2))  # (128, npat, pw)
    else:
        pat = None
    _program_cache[key] = (nc, pat)
    return nc, pat


def _pack_x(xT: np.ndarray) -> np.ndarray:
    # x^T (D, S) -> (128, NQB, DT, 512): [p, c, dt, j] = x^T[dt*128+p, c*512+j]
    return np.ascontiguousarray(
        xT.reshape(DT, 128, NQB, 512).transpose(1, 2, 0, 3))


def _pack_v(xT: np.ndarray) -> np.ndarray:
    # x^T (D, S) -> (128, NQB, 4, DT, 128): per-128-token-tile granularity
    return np.ascontiguousarray(
        xT.reshape(DT, 128, NQB, 4, 128).transpose(1, 2, 3, 0, 4))


def _pack_w(wT: np.ndarray) -> np.ndarray:
    # W^T slice (D, FPC) -> (128, DT, FPC)
    return np.ascontiguousarray(wT.reshape(DT, 128, FPC).transpose(1, 0, 2))


def kernel(**inputs) -> np.ndarray:
    global LAST_RESULT
    query = np.asarray(inputs["query"], np.float32)
    key = np.asarray(inputs["key"], np.float32)
    value = np.asarray(inputs["value"], np.float32)
    mask = np.asarray(inputs["mask"])
    Wq = np.asarray(inputs["Wq"], np.float32)
    bq = np.asarray(inputs["bq"], np.float32)
    Wk = np.asarray(inputs["Wk"], np.float32)
    bk = np.asarray(inputs["bk"], np.float32)
    Wv = np.asarray(inputs["Wv"], np.float32)
    bv = np.asarray(inputs["bv"], np.float32)
    Wo = np.asarray(inputs["Wo"], np.float32)
    bo = np.asarray(inputs["bo"], np.float32)

    nc, pat = _get_program(mask.reshape(S, S))

    WqT, WkT, WvT, WoT = Wq.T, Wk.T, Wv.T, Wo.T
    # packed per-batch inputs, shared across the 4 cores of a batch
    xp = {}
    for t, x in (("q", query), ("k", key)):
        xp[t] = [_pack_x(np.ascontiguousarray(x[b].T).astype(BFNP))
                 for b in range(B)]
    xp["v"] = [_pack_v(np.ascontiguousarray(value[b].T).astype(BFNP))
               for b in range(B)]
    xkq_b = [np.ascontiguousarray(
        np.stack([xp["k"][b], xp["q"][b]], axis=2)) for b in range(B)]
    in_maps = []
    for c in range(NCORE):
        b, g = divmod(c, GROUPS)
        f0 = g * FPC
        wall = np.stack([
            _pack_w(WkT[:, f0:f0 + FPC].astype(BFNP)),
            _pack_w(WqT[:, f0:f0 + FPC].astype(BFNP)),
            _pack_w(WvT[:, f0:f0 + FPC].astype(BFNP)),
        ], axis=1)  # (128, 3, DT, FPC)
        wop = np.ascontiguousarray(
            WoT[f0:f0 + FPC, :].astype(BFNP).reshape(FT, 128, D)
            .transpose(1, 0, 2))  # (128, FT, D)
        bqkp = np.ascontiguousarray(
            np.stack([bq[f0:f0 + FPC].reshape(FT, 128),
                      bk[f0:f0 + FPC].reshape(FT, 128)])
            .transpose(2, 0, 1).astype(np.float32))  # (128, 2, FT)
        m = {"xkq": xkq_b[b], "xv4": xp["v"][b], "wall": wall,
             "wo": wop, "bqk": bqkp}
        if pat is not None:
            m["masks"] = pat
        in_maps.append(m)

    res = run_bass_kernel_spmd(
        nc, in_maps, core_ids=list(range(NCORE)),
        trace=PROFILE,
        trace_cores=(TRACE_CORES if TRACE_CORES is not None
                     else (list(range(NCORE)) if PROFILE else None)),
    )
    LAST_RESULT = res

    host_bias = bo + bv @ WoT  # (D,) folded V/O biases, added once per batch
    out = np.empty((B, S, D), np.float32)
    for b in range(B):
        acc = res.results[b * GROUPS]["out"].astype(np.float32)
        for g in range(1, GROUPS):
            acc = acc + res.results[b * GROUPS + g]["out"].astype(np.float32)
        out[b] = acc + host_bias
    return out


# revision 22
# speedup vs baseline: 1.0164x; 1.0164x over previous
"""Multi-head attention (B=2, S=2048, D=1024, H=16) on 8 Trainium2 NeuronCores.

Sharding: core c -> (batch b = c//4, head-group g = c%4).  Each core computes
Q/K/V projections for its 4 heads (256 features), causal attention for those
heads over the full sequence, and a partial O-projection (its 256 attn
features x full Wo.T slice).  The host sums the 4 partial outputs per batch
and folds in the biases that commute with the reduction (bo, bv @ Wo.T).

Device-side layout (per core, all matmul operands bf16, accumulation f32):
  Q^T, K^T  [feat, tok]   (feature-on-partition; per-partition bias on DVE)
  V         [tok, feat+1] (augmented with a ones column -> PV matmul also
                           accumulates the softmax denominator)
  scores^T  [k, q] tiles  -> exp on ScalarE with fused 1/sqrt(dk) scale; no
                           max-subtraction (scores are O(5) for this data,
                           exp is exact to 2 ULP and f32 can't overflow).
                           Two k-tiles share one [128,2,512] PSUM tile so a
                           single ACTIVATE covers both.
  masking   multiplicative bf16 tiles after exp; partially-masked tiles also
            carry a start column c0 so QK/exp/PV skip the dead q-range
  attnU^T + denom = V_aug^T @ P^T accumulated over k tiles in PSUM
  normalize: DVE reciprocal_approx_fast + GpSimd partition-broadcast + DVE mul
  O-proj    attn^T tiles stationary, Wo^T slice streaming -> partial out
            bf16, drained per 128-token tile and DMA'd out immediately

Software pipeline: the kernel runs `proj(0); for qb: attention(qb),
proj(qb+1), oproj(qb)`.  proj(qb+1) sits between attention(qb) and
oproj(qb) in program order so the Tile scheduler fills TensorE stalls
(waiting on ScalarE exp) with projection matmuls, and the shared psacc
PSUM pool's rotation never chains a projection group behind an
attention-dependent O-projection group.

All inputs are host-packed into SBUF layout (partition-major, contiguous
per partition) so each input DMA is 128 fat descriptors: descriptor
generation cost ~1.3us/transfer instead of ~3us, and the per-query-block
K/Q/V chunks arrive in consumption order on one FIFO queue.
"""

import hashlib
from contextlib import ExitStack

import ml_dtypes
import numpy as np

import concourse.bass as bass
import concourse.tile as tile
from concourse import bacc, mybir
from concourse.bass_utils import run_bass_kernel_spmd

B, S, D, H = 2, 2048, 1024, 16
DK = D // H                  # 64 head dim
NCORE = 8
GROUPS = NCORE // B          # 4 head-groups per batch
HPC = H // GROUPS            # 4 heads per core
FPC = HPC * DK               # 256 features per core
FT = FPC // 128              # 2 feature tiles per core
DT = D // 128                # 8 d_in tiles
TT = S // 128                # 16 token tiles (k tiles)
QB = 512                     # query block (free-dim) size in attention
NQB = S // QB                # 4 query blocks
NCH = 512                    # psum free-dim chunk for projections
BF = mybir.dt.bfloat16
F32 = mybir.dt.float32
BFNP = ml_dtypes.bfloat16

# module-level knobs for test.py
PROFILE = False
TRACE_CORES = None
LAST_RESULT = None

_program_cache: dict = {}


def _classify_mask(mask2d: np.ndarray):
    """Classify (S, S) keep-mask into per-(qblock, ktile) modes.

    Returns (plan, patterns): plan[qb] is a list of (kt, mask_id|None, c0)
    for tiles that are at least partially kept, where c0 is the first
    q-column (within the block) with any kept key; patterns is a list of
    [128, QB] bf16 multiplicative mask tiles (k on partitions, q free).
    """
    keep = np.asarray(mask2d) != 0
    patterns = []
    pattern_ids = {}
    plan = []
    for qb in range(NQB):
        row = []
        for kt in range(TT):
            blk = keep[qb * QB:(qb + 1) * QB, kt * 128:(kt + 1) * 128].T
            if not blk.any():
                continue
            if blk.all():
                row.append((kt, None, 0, 0))
                continue
            # c0: first column with any kept key (QK/exp/PV start here).
            # c1: first column from which every column is all-keep; only
            # [c0, c1) needs the multiplicative mask.
            anyk = blk.any(axis=0)
            allk = blk.all(axis=0)
            c0 = int(np.flatnonzero(anyk)[0])
            notall = np.flatnonzero(~allk)
            c1 = int(notall[-1]) + 1 if notall.size else c0
            pat = blk[:, c0:c1]
            key = pat.tobytes()
            mid = pattern_ids.get(key)
            if mid is None:
                mid = len(patterns)
                pattern_ids[key] = mid
                patterns.append(pat.astype(BFNP))
            row.append((kt, mid, c0, c1))
        plan.append(row)
    return plan, patterns


def build_program(plan, npat, pw, combos):
    nc = bacc.Bacc("TRN2", target_bir_lowering=False, debug=False,
                   num_devices=NCORE)
    # host-packed inputs: partition-major, contiguous per partition
    xkq = nc.dram_tensor("xkq", (128, NQB, 2, DT, 512), BF,
                         kind="ExternalInput").ap()
    xv4 = nc.dram_tensor("xv4", (128, NQB, 4, DT, 128), BF,
                         kind="ExternalInput").ap()
    wall = nc.dram_tensor("wall", (128, 3, DT, FPC), BF,
                          kind="ExternalInput").ap()
    wo = nc.dram_tensor("wo", (128, FT, D), BF, kind="ExternalInput").ap()
    bqk = nc.dram_tensor("bqk", (128, 2, FT), F32, kind="ExternalInput").ap()
    masks = None
    if npat:
        masks = nc.dram_tensor("masks", (128, npat, pw), BF,
                               kind="ExternalInput").ap()
    out = nc.dram_tensor("out", (S, D), BF, kind="ExternalOutput").ap()
    out_r = out.rearrange("(t p) d -> t p d", p=128)

    with tile.TileContext(nc) as tc, ExitStack() as ctx:
        singles = ctx.enter_context(tc.tile_pool(name="singles", bufs=1))
        xpool = ctx.enter_context(tc.tile_pool(name="xpool", bufs=2))
        ppool = ctx.enter_context(tc.tile_pool(name="ppool", bufs=6))
        npool = ctx.enter_context(tc.tile_pool(name="npool", bufs=3))
        opool = ctx.enter_context(tc.tile_pool(name="opool", bufs=3))
        psacc = ctx.enter_context(tc.tile_pool(name="psacc", bufs=2, space="PSUM"))
        psS = ctx.enter_context(tc.tile_pool(name="psS", bufs=2, space="PSUM"))
        psPV = ctx.enter_context(tc.tile_pool(name="psPV", bufs=2, space="PSUM"))

        # ---- SBUF residents ----
        w_sb = singles.tile([128, 3, DT, FPC], BF)     # wk | wq | wv
        wo_sb = singles.tile([128, FT, D], BF)
        bias_sb = singles.tile([128, 2, FT], F32)
        mask_sb = None
        if npat:
            mask_sb = singles.tile([128, npat, pw], BF, name="mask_sb")
        q_sb = singles.tile([128, FT, S], BF)
        k_sb = singles.tile([128, FT, S], BF)
        attn_sb = singles.tile([128, FT, S], BF)
        v_sb = singles.tile([128, TT, HPC, DK + 1], BF)
        # per-chunk k|q and v input tiles; bufs=2 so the dma_start for
        # chunk c waits for chunk c-2's last projection read -> later
        # chunks don't steal DMA bandwidth from earlier-needed data
        x_ch = [xpool.tile([128, 2, DT, 512], BF, tag="xkq", name=f"x_ch{c}")
                for c in range(NQB)]
        xv_ch = [xpool.tile([128, 4, DT, 128], BF, tag="xv", name=f"xv_ch{c}")
                 for c in range(NQB)]

        # ---- DMA issue order == consumption order, one FIFO queue.
        # V weights + first token tiles lead so TensorE can start V
        # projections ~10us in, while the K/Q chunk data still streams. ----
        nc.sync.dma_start(bias_sb, bqk)
        nc.sync.dma_start(w_sb[:, 2], wall[:, 2])
        nc.sync.dma_start(xv_ch[0][:, 0:2], xv4[:, 0, 0:2])
        nc.sync.dma_start(w_sb[:, 0], wall[:, 0])
        nc.sync.dma_start(x_ch[0][:, 0], xkq[:, 0, 0])
        nc.sync.dma_start(xv_ch[0][:, 2:4], xv4[:, 0, 2:4])
        nc.sync.dma_start(w_sb[:, 1], wall[:, 1])
        nc.sync.dma_start(x_ch[0][:, 1], xkq[:, 0, 1])
        if npat:
            nc.sync.dma_start(mask_sb, masks)
        # dummy readback of chunk 0: stalls the sync DGE queue until chunk 0
        # has landed, so chunk 1's transfer can't steal DMA bandwidth from
        # the data the first projections are waiting on.  The target rows of
        # `out` are overwritten by the real output DMAs later on this queue.
        nc.sync.dma_start(out[0:1, 0:64], x_ch[0][0:1, 1, DT - 1, 448:])
        nc.sync.dma_start(x_ch[1], xkq[:, 1])
        nc.sync.dma_start(xv_ch[1], xv4[:, 1])
        nc.sync.dma_start(wo_sb, wo)
        for c in range(2, NQB):
            nc.sync.dma_start(x_ch[c], xkq[:, c])
            nc.sync.dma_start(xv_ch[c], xv4[:, c])

        for tt in range(TT):
            nc.vector.memset(v_sb[:, tt, :, DK:DK + 1], 1.0)

        def proj_kq(which, nch):
            # K^T / Q^T projection chunk ([feat, tok] layout, bias on DVE)
            xi, wi, y_sb, bi = ((0, 0, k_sb, 1) if which == "k"
                                else (1, 1, q_sb, 0))
            for ft in range(FT):
                ps = psacc.tile([128, NCH], F32, tag="acc")
                for dt in range(DT):
                    nc.tensor.matmul(
                        ps,
                        lhsT=w_sb[:, wi, dt, ft * 128:(ft + 1) * 128],
                        rhs=x_ch[nch][:, xi, dt, :],
                        start=(dt == 0), stop=(dt == DT - 1))
                nc.vector.tensor_scalar_add(
                    y_sb[:, ft, nch * NCH:(nch + 1) * NCH], ps,
                    bias_sb[:, bi, ft:ft + 1])

        def proj_v(tt):
            # V projection token tile (natural layout [tok, feat])
            c, r = divmod(tt, 4)
            ps = psacc.tile([128, FPC], F32, tag="acc")
            for dt in range(DT):
                nc.tensor.matmul(ps,
                                 lhsT=xv_ch[c][:, r, dt, :],
                                 rhs=w_sb[:, 2, dt, :],
                                 start=(dt == 0), stop=(dt == DT - 1))
            nc.vector.tensor_copy(v_sb[:, tt, :, 0:DK],
                                  ps.rearrange("p (h d) -> p h d", h=HPC))

        def proj(qb):
            # qb=0 runs V first: its data leads the DMA stream, so TensorE
            # has work while the K/Q chunks are still arriving
            if qb == 0:
                for tt in range(4):
                    proj_v(tt)
            proj_kq("k", qb)
            proj_kq("q", qb)
            if qb > 0:
                for tt in range(qb * 4, qb * 4 + 4):
                    proj_v(tt)

        inv_sqrt_dk = float(1.0 / np.sqrt(DK))

        def attention(qb):
            kts = plan[qb]
            for h in range(HPC):
                if not kts:
                    continue
                pr = (h % 2) * 64
                ft = h // 2
                pairs = [kts[i:i + 2] for i in range(0, len(kts), 2)]
                pv = psPV.tile([DK + 1, QB], F32)
                n = 0
                for pair in pairs:
                    pc0 = min(c0 for (_, _, c0, _) in pair)
                    s_ps = psS.tile([128, 2, QB], F32)
                    for j, (kt, _, c0, _) in enumerate(pair):
                        nc.tensor.matmul(
                            s_ps[:, j, c0:],
                            lhsT=k_sb[pr:pr + DK, ft, kt * 128:(kt + 1) * 128],
                            rhs=q_sb[pr:pr + DK, ft,
                                     qb * QB + c0:(qb + 1) * QB],
                            start=True, stop=True)
                    pT = ppool.tile([128, 2, QB], BF, tag="pt")
                    nc.scalar.activation(pT[:, 0:len(pair), pc0:],
                                         s_ps[:, 0:len(pair), pc0:],
                                         mybir.ActivationFunctionType.Exp,
                                         scale=inv_sqrt_dk)
                    combo = None
                    if len(pair) == 2:
                        (k0, m0, a0, b0), (k1, m1, a1, b1) = pair
                        if m0 is not None and m1 is not None:
                            combo = combos.get((m0, m1, b0 - a0, b1 - a1))
                    if combo is not None:
                        cmid, w = combo
                        assert mask_sb is not None
                        sl = pT[:, 0, a0:a0 + w]
                        both = bass.AP(
                            tensor=sl.tensor, offset=sl.offset,
                            ap=[sl.ap[0], [QB + (a1 - a0), 2], sl.ap[-1]])
                        nc.vector.tensor_mul(
                            both, both,
                            mask_sb[:, cmid, 0:2 * w].rearrange(
                                "p (j w) -> p j w", j=2))
                    else:
                        for j, (kt, mid, c0, c1) in enumerate(pair):
                            if mid is not None and c1 > c0:
                                assert mask_sb is not None
                                nc.vector.tensor_mul(
                                    pT[:, j, c0:c1], pT[:, j, c0:c1],
                                    mask_sb[:, mid, 0:c1 - c0])
                    for j, (kt, _, c0, _) in enumerate(pair):
                        nc.tensor.matmul(pv[:, c0:], lhsT=v_sb[:, kt, h, :],
                                         rhs=pT[:, j, c0:],
                                         start=(n == 0),
                                         stop=(n == len(kts) - 1))
                        n += 1
                # normalize: attn^T[d, q] = attnU^T[d, q] / denom[q]
                # NOTE: reciprocal_approx_fast (custom DVE op) reading PSUM
                # directly returns garbage on HW -- copy to SBUF first.
                den = npool.tile([1, QB], F32, tag="den")
                nc.vector.tensor_copy(den, pv[DK:DK + 1, :])
                rec = npool.tile([1, QB], F32, tag="rec")
                nc.vector.reciprocal_approx_fast(rec, den)
                bc = npool.tile([64, QB], F32, tag="bc")
                nc.gpsimd.partition_broadcast(bc, rec)
                dst = attn_sb[pr:pr + DK, ft, qb * QB:(qb + 1) * QB]
                nc.vector.tensor_mul(dst, pv[0:DK, :], bc)

        def oproj(qb):
            # partial O-projection; bf16 staging per 128-token tile, DMA'd
            # out immediately so the final block's tail is short
            for qt in range(qb * QB // 128, (qb + 1) * QB // 128):
                ob = opool.tile([128, D], BF)
                for nch in range(D // NCH):
                    ps = psacc.tile([128, NCH], F32, tag="acc")
                    for hd in range(FT):
                        nc.tensor.matmul(
                            ps,
                            lhsT=attn_sb[:, hd, qt * 128:(qt + 1) * 128],
                            rhs=wo_sb[:, hd, nch * NCH:(nch + 1) * NCH],
                            start=(hd == 0), stop=(hd == FT - 1))
                    nc.vector.tensor_copy(ob[:, nch * NCH:(nch + 1) * NCH], ps)
                nc.sync.dma_start(out_r[qt], ob)

        # ---- software-pipelined main loop ----
        proj(0)
        for qb in range(NQB):
            attention(qb)
            if qb + 1 < NQB:
                proj(qb + 1)
            oproj(qb)

    nc.compile()
    return nc


def _get_program(mask2d: np.ndarray):
    key = hashlib.sha1(np.ascontiguousarray(mask2d).tobytes()).hexdigest()
    hit = _program_cache.get(key)
    if hit is not None:
        return hit
    plan, patterns = _classify_mask(mask2d)
    # combine the two masked tiles of a k-tile pair into one [128, 2, w]
    # pattern so a single strided DVE multiply covers both strips
    combos = {}
    for row in plan:
        for i in range(0, len(row) - 1, 2):
            (k0, m0, a0, b0), (k1, m1, a1, b1) = row[i], row[i + 1]
            if m0 is None or m1 is None:
                continue
            w = max(b0 - a0, b1 - a1)
            if a0 + w > QB or a1 + w > QB or w <= 0 or w > 128:
                continue
            key = (m0, m1, b0 - a0, b1 - a1)
            if key in combos:
                continue
            cp = np.ones((128, 2, w), BFNP)
            cp[:, 0, :b0 - a0] = patterns[m0]
            cp[:, 1, :b1 - a1] = patterns[m1]
            combos[key] = (len(patterns), w)
            patterns.append(cp.reshape(128, 2 * w))
    pw = max((p.shape[1] for p in patterns), default=0)
    nc = build_program(plan, len(patterns), pw, combos)
    if patterns:
        pat = np.zeros((len(patterns), 128, pw), BFNP)
        for i, p in enumerate(patterns):
            pat[i, :, :p.shape[1]] = p
        pat = np.ascontiguousarray(pat.transpose(1, 0, 2))  # (128, npat, pw)
    else:
        pat = None
    _program_cache[key] = (nc, pat)
    return nc, pat


def _pack_x(xT: np.ndarray) -> np.ndarray:
    # x^T (D, S) -> (128, NQB, DT, 512): [p, c, dt, j] = x^T[dt*128+p, c*512+j]
    return np.ascontiguousarray(
        xT.reshape(DT, 128, NQB, 512).transpose(1, 2, 0, 3))


def _pack_v(xT: np.ndarray) -> np.ndarray:
    # x^T (D, S) -> (128, NQB, 4, DT, 128): per-128-token-tile granularity
    return np.ascontiguousarray(
        xT.reshape(DT, 128, NQB, 4, 128).transpose(1, 2, 3, 0, 4))


def _pack_w(wT: np.ndarray) -> np.ndarray:
    # W^T slice (D, FPC) -> (128, DT, FPC)
    return np.ascontiguousarray(wT.reshape(DT, 128, FPC).transpose(1, 0, 2))


def kernel(**inputs) -> np.ndarray:
    global LAST_RESULT
    query = np.asarray(inputs["query"], np.float32)
    key = np.asarray(inputs["key"], np.float32)
    value = np.asarray(inputs["value"], np.float32)
    mask = np.asarray(inputs["mask"])
    Wq = np.asarray(inputs["Wq"], np.float32)
    bq = np.asarray(inputs["bq"], np.float32)
    Wk = np.asarray(inputs["Wk"], np.float32)
    bk = np.asarray(inputs["bk"], np.float32)
    Wv = np.asarray(inputs["Wv"], np.float32)
    bv = np.asarray(inputs["bv"], np.float32)
    Wo = np.asarray(inputs["Wo"], np.float32)
    bo = np.asarray(inputs["bo"], np.float32)

    nc, pat = _get_program(mask.reshape(S, S))

    WqT, WkT, WvT, WoT = Wq.T, Wk.T, Wv.T, Wo.T
    # packed per-batch inputs, shared across the 4 cores of a batch
    xp = {}
    for t, x in (("q", query), ("k", key)):
        xp[t] = [_pack_x(np.ascontiguousarray(x[b].T).astype(BFNP))
                 for b in range(B)]
    xp["v"] = [_pack_v(np.ascontiguousarray(value[b].T).astype(BFNP))
               for b in range(B)]
    xkq_b = [np.ascontiguousarray(
        np.stack([xp["k"][b], xp["q"][b]], axis=2)) for b in range(B)]
    in_maps = []
    for c in range(NCORE):
        b, g = divmod(c, GROUPS)
        f0 = g * FPC
        wall = np.stack([
            _pack_w(WkT[:, f0:f0 + FPC].astype(BFNP)),
            _pack_w(WqT[:, f0:f0 + FPC].astype(BFNP)),
            _pack_w(WvT[:, f0:f0 + FPC].astype(BFNP)),
        ], axis=1)  # (128, 3, DT, FPC)
        wop = np.ascontiguousarray(
            WoT[f0:f0 + FPC, :].astype(BFNP).reshape(FT, 128, D)
            .transpose(1, 0, 2))  # (128, FT, D)
        bqkp = np.ascontiguousarray(
            np.stack([bq[f0:f0 + FPC].reshape(FT, 128),
                      bk[f0:f0 + FPC].reshape(FT, 128)])
            .transpose(2, 0, 1).astype(np.float32))  # (128, 2, FT)
        m = {"xkq": xkq_b[b], "xv4": xp["v"][b], "wall": wall,
             "wo": wop, "bqk": bqkp}
        if pat is not None:
            m["masks"] = pat
        in_maps.append(m)

    res = run_bass_kernel_spmd(
        nc, in_maps, core_ids=list(range(NCORE)),
        trace=PROFILE,
        trace_cores=(TRACE_CORES if TRACE_CORES is not None
                     else (list(range(NCORE)) if PROFILE else None)),
    )
    LAST_RESULT = res

    host_bias = bo + bv @ WoT  # (D,) folded V/O biases, added once per batch
    out = np.empty((B, S, D), np.float32)
    for b in range(B):
        acc = res.results[b * GROUPS]["out"].astype(np.float32)
        for g in range(1, GROUPS):
            acc = acc + res.results[b * GROUPS + g]["out"].astype(np.float32)
        out[b] = acc + host_bias
    return out


# revision 23
# speedup vs baseline: 1.0175x; 1.0011x over previous
"""Multi-head attention (B=2, S=2048, D=1024, H=16) on 8 Trainium2 NeuronCores.

Sharding: core c -> (batch b = c//4, head-group g = c%4).  Each core computes
Q/K/V projections for its 4 heads (256 features), causal attention for those
heads over the full sequence, and a partial O-projection (its 256 attn
features x full Wo.T slice).  The host sums the 4 partial outputs per batch
and folds in the biases that commute with the reduction (bo, bv @ Wo.T).

Device-side layout (per core, all matmul operands bf16, accumulation f32):
  Q^T, K^T  [feat, tok]   (feature-on-partition; per-partition bias on DVE)
  V         [tok, feat+1] (augmented with a ones column -> PV matmul also
                           accumulates the softmax denominator)
  scores^T  [k, q] tiles  -> exp on ScalarE with fused 1/sqrt(dk) scale; no
                           max-subtraction (scores are O(5) for this data,
                           exp is exact to 2 ULP and f32 can't overflow).
                           Two k-tiles share one [128,2,512] PSUM tile so a
                           single ACTIVATE covers both.
  masking   multiplicative bf16 tiles after exp; partially-masked tiles also
            carry a start column c0 so QK/exp/PV skip the dead q-range
  attnU^T + denom = V_aug^T @ P^T accumulated over k tiles in PSUM
  normalize: DVE reciprocal_approx_fast + GpSimd partition-broadcast + DVE mul
  O-proj    attn^T tiles stationary, Wo^T slice streaming -> partial out
            bf16, drained per 128-token tile and DMA'd out immediately

Software pipeline: the kernel runs `proj(0); for qb: attention(qb),
proj(qb+1), oproj(qb)`.  proj(qb+1) sits between attention(qb) and
oproj(qb) in program order so the Tile scheduler fills TensorE stalls
(waiting on ScalarE exp) with projection matmuls, and the shared psacc
PSUM pool's rotation never chains a projection group behind an
attention-dependent O-projection group.

All inputs are host-packed into SBUF layout (partition-major, contiguous
per partition) so each input DMA is 128 fat descriptors: descriptor
generation cost ~1.3us/transfer instead of ~3us, and the per-query-block
K/Q/V chunks arrive in consumption order on one FIFO queue.
"""

import hashlib
from contextlib import ExitStack

import ml_dtypes
import numpy as np

import concourse.bass as bass
import concourse.tile as tile
from concourse import bacc, mybir
from concourse.bass_utils import run_bass_kernel_spmd

B, S, D, H = 2, 2048, 1024, 16
DK = D // H                  # 64 head dim
NCORE = 8
GROUPS = NCORE // B          # 4 head-groups per batch
HPC = H // GROUPS            # 4 heads per core
FPC = HPC * DK               # 256 features per core
FT = FPC // 128              # 2 feature tiles per core
DT = D // 128                # 8 d_in tiles
TT = S // 128                # 16 token tiles (k tiles)
QB = 512                     # query block (free-dim) size in attention
NQB = S // QB                # 4 query blocks
NCH = 512                    # psum free-dim chunk for projections
BF = mybir.dt.bfloat16
F32 = mybir.dt.float32
BFNP = ml_dtypes.bfloat16

# module-level knobs for test.py
PROFILE = False
TRACE_CORES = None
LAST_RESULT = None

_program_cache: dict = {}


def _classify_mask(mask2d: np.ndarray):
    """Classify (S, S) keep-mask into per-(qblock, ktile) modes.

    Returns (plan, patterns): plan[qb] is a list of (kt, mask_id|None, c0)
    for tiles that are at least partially kept, where c0 is the first
    q-column (within the block) with any kept key; patterns is a list of
    [128, QB] bf16 multiplicative mask tiles (k on partitions, q free).
    """
    keep = np.asarray(mask2d) != 0
    patterns = []
    pattern_ids = {}
    plan = []
    for qb in range(NQB):
        row = []
        for kt in range(TT):
            blk = keep[qb * QB:(qb + 1) * QB, kt * 128:(kt + 1) * 128].T
            if not blk.any():
                continue
            if blk.all():
                row.append((kt, None, 0, 0))
                continue
            # c0: first column with any kept key (QK/exp/PV start here).
            # c1: first column from which every column is all-keep; only
            # [c0, c1) needs the multiplicative mask.
            anyk = blk.any(axis=0)
            allk = blk.all(axis=0)
            c0 = int(np.flatnonzero(anyk)[0])
            notall = np.flatnonzero(~allk)
            c1 = int(notall[-1]) + 1 if notall.size else c0
            pat = blk[:, c0:c1]
            key = pat.tobytes()
            mid = pattern_ids.get(key)
            if mid is None:
                mid = len(patterns)
                pattern_ids[key] = mid
                patterns.append(pat.astype(BFNP))
            row.append((kt, mid, c0, c1))
        plan.append(row)
    return plan, patterns


def build_program(plan, npat, pw, combos):
    nc = bacc.Bacc("TRN2", target_bir_lowering=False, debug=False,
                   num_devices=NCORE)
    # host-packed inputs: partition-major, contiguous per partition
    xkq = nc.dram_tensor("xkq", (128, NQB, 2, DT, 512), BF,
                         kind="ExternalInput").ap()
    xv4 = nc.dram_tensor("xv4", (128, NQB, 4, DT, 128), BF,
                         kind="ExternalInput").ap()
    wall = nc.dram_tensor("wall", (128, 3, DT, FPC), BF,
                          kind="ExternalInput").ap()
    wo = nc.dram_tensor("wo", (128, FT, D), BF, kind="ExternalInput").ap()
    bqk = nc.dram_tensor("bqk", (128, 2, FT), F32, kind="ExternalInput").ap()
    masks = None
    if npat:
        masks = nc.dram_tensor("masks", (128, npat, pw), BF,
                               kind="ExternalInput").ap()
    out = nc.dram_tensor("out", (S, D), BF, kind="ExternalOutput").ap()
    out_r = out.rearrange("(t p) d -> t p d", p=128)

    with tile.TileContext(nc) as tc, ExitStack() as ctx:
        singles = ctx.enter_context(tc.tile_pool(name="singles", bufs=1))
        xpool = ctx.enter_context(tc.tile_pool(name="xpool", bufs=2))
        ppool = ctx.enter_context(tc.tile_pool(name="ppool", bufs=6))
        npool = ctx.enter_context(tc.tile_pool(name="npool", bufs=3))
        opool = ctx.enter_context(tc.tile_pool(name="opool", bufs=3))
        psacc = ctx.enter_context(tc.tile_pool(name="psacc", bufs=2, space="PSUM"))
        psS = ctx.enter_context(tc.tile_pool(name="psS", bufs=2, space="PSUM"))
        psPV = ctx.enter_context(tc.tile_pool(name="psPV", bufs=2, space="PSUM"))

        # ---- SBUF residents ----
        w_sb = singles.tile([128, 3, DT, FPC], BF)     # wk | wq | wv
        wo_sb = singles.tile([128, FT, D], BF)
        bias_sb = singles.tile([128, 2, FT], F32)
        mask_sb = None
        if npat:
            mask_sb = singles.tile([128, npat, pw], BF, name="mask_sb")
        q_sb = singles.tile([128, FT, S], BF)
        k_sb = singles.tile([128, FT, S], BF)
        attn_sb = singles.tile([128, FT, S], BF)
        v_sb = singles.tile([128, TT, HPC, DK + 1], BF)
        # per-chunk k|q and v input tiles; bufs=2 so the dma_start for
        # chunk c waits for chunk c-2's last projection read -> later
        # chunks don't steal DMA bandwidth from earlier-needed data
        x_ch = [xpool.tile([128, 2, DT, 512], BF, tag="xkq", name=f"x_ch{c}")
                for c in range(NQB)]
        xv_ch = [xpool.tile([128, 4, DT, 128], BF, tag="xv", name=f"xv_ch{c}")
                 for c in range(NQB)]

        # ---- DMA issue order == consumption order, one FIFO queue.
        # V weights + first token tiles lead so TensorE can start V
        # projections ~10us in, while the K/Q chunk data still streams. ----
        nc.sync.dma_start(bias_sb, bqk)
        nc.sync.dma_start(w_sb[:, 2], wall[:, 2])
        nc.sync.dma_start(xv_ch[0][:, 0:2], xv4[:, 0, 0:2])
        nc.sync.dma_start(w_sb[:, 0], wall[:, 0])
        nc.sync.dma_start(x_ch[0][:, 0, 0:4], xkq[:, 0, 0, 0:4])
        nc.sync.dma_start(xv_ch[0][:, 2:4], xv4[:, 0, 2:4])
        nc.sync.dma_start(x_ch[0][:, 0, 4:8], xkq[:, 0, 0, 4:8])
        nc.sync.dma_start(w_sb[:, 1], wall[:, 1])
        nc.sync.dma_start(x_ch[0][:, 1, 0:4], xkq[:, 0, 1, 0:4])
        nc.sync.dma_start(x_ch[0][:, 1, 4:8], xkq[:, 0, 1, 4:8])
        if npat:
            nc.sync.dma_start(mask_sb, masks)
        # dummy readback of chunk 0: stalls the sync DGE queue until chunk 0
        # has landed, so chunk 1's transfer can't steal DMA bandwidth from
        # the data the first projections are waiting on.  The target rows of
        # `out` are overwritten by the real output DMAs later on this queue.
        nc.sync.dma_start(out[0:1, 0:64], x_ch[0][0:1, 1, DT - 1, 448:])
        nc.sync.dma_start(x_ch[1], xkq[:, 1])
        nc.sync.dma_start(xv_ch[1], xv4[:, 1])
        nc.sync.dma_start(wo_sb, wo)
        for c in range(2, NQB):
            nc.sync.dma_start(x_ch[c], xkq[:, c])
            nc.sync.dma_start(xv_ch[c], xv4[:, c])

        for tt in range(TT):
            nc.vector.memset(v_sb[:, tt, :, DK:DK + 1], 1.0)

        def proj_kq(which, nch):
            # K^T / Q^T projection chunk ([feat, tok] layout, bias on DVE)
            xi, wi, y_sb, bi = ((0, 0, k_sb, 1) if which == "k"
                                else (1, 1, q_sb, 0))
            for ft in range(FT):
                ps = psacc.tile([128, NCH], F32, tag="acc")
                for dt in range(DT):
                    nc.tensor.matmul(
                        ps,
                        lhsT=w_sb[:, wi, dt, ft * 128:(ft + 1) * 128],
                        rhs=x_ch[nch][:, xi, dt, :],
                        start=(dt == 0), stop=(dt == DT - 1))
                nc.vector.tensor_scalar_add(
                    y_sb[:, ft, nch * NCH:(nch + 1) * NCH], ps,
                    bias_sb[:, bi, ft:ft + 1])

        def proj_v(tt):
            # V projection token tile (natural layout [tok, feat])
            c, r = divmod(tt, 4)
            ps = psacc.tile([128, FPC], F32, tag="acc")
            for dt in range(DT):
                nc.tensor.matmul(ps,
                                 lhsT=xv_ch[c][:, r, dt, :],
                                 rhs=w_sb[:, 2, dt, :],
                                 start=(dt == 0), stop=(dt == DT - 1))
            nc.vector.tensor_copy(v_sb[:, tt, :, 0:DK],
                                  ps.rearrange("p (h d) -> p h d", h=HPC))

        def proj(qb):
            # qb=0 runs V first: its data leads the DMA stream, so TensorE
            # has work while the K/Q chunks are still arriving
            if qb == 0:
                for tt in range(4):
                    proj_v(tt)
            proj_kq("k", qb)
            proj_kq("q", qb)
            if qb > 0:
                for tt in range(qb * 4, qb * 4 + 4):
                    proj_v(tt)

        inv_sqrt_dk = float(1.0 / np.sqrt(DK))

        def attention(qb):
            kts = plan[qb]
            for h in range(HPC):
                if not kts:
                    continue
                pr = (h % 2) * 64
                ft = h // 2
                pairs = [kts[i:i + 2] for i in range(0, len(kts), 2)]
                pv = psPV.tile([DK + 1, QB], F32)
                n = 0
                for pair in pairs:
                    pc0 = min(c0 for (_, _, c0, _) in pair)
                    s_ps = psS.tile([128, 2, QB], F32)
                    for j, (kt, _, c0, _) in enumerate(pair):
                        nc.tensor.matmul(
                            s_ps[:, j, c0:],
                            lhsT=k_sb[pr:pr + DK, ft, kt * 128:(kt + 1) * 128],
                            rhs=q_sb[pr:pr + DK, ft,
                                     qb * QB + c0:(qb + 1) * QB],
                            start=True, stop=True)
                    pT = ppool.tile([128, 2, QB], BF, tag="pt")
                    nc.scalar.activation(pT[:, 0:len(pair), pc0:],
                                         s_ps[:, 0:len(pair), pc0:],
                                         mybir.ActivationFunctionType.Exp,
                                         scale=inv_sqrt_dk)
                    combo = None
                    if len(pair) == 2:
                        (k0, m0, a0, b0), (k1, m1, a1, b1) = pair
                        if m0 is not None and m1 is not None:
                            combo = combos.get((m0, m1, b0 - a0, b1 - a1))
                    if combo is not None:
                        cmid, w = combo
                        assert mask_sb is not None
                        sl = pT[:, 0, a0:a0 + w]
                        both = bass.AP(
                            tensor=sl.tensor, offset=sl.offset,
                            ap=[sl.ap[0], [QB + (a1 - a0), 2], sl.ap[-1]])
                        nc.vector.tensor_mul(
                            both, both,
                            mask_sb[:, cmid, 0:2 * w].rearrange(
                                "p (j w) -> p j w", j=2))
                    else:
                        for j, (kt, mid, c0, c1) in enumerate(pair):
                            if mid is not None and c1 > c0:
                                assert mask_sb is not None
                                nc.vector.tensor_mul(
                                    pT[:, j, c0:c1], pT[:, j, c0:c1],
                                    mask_sb[:, mid, 0:c1 - c0])
                    for j, (kt, _, c0, _) in enumerate(pair):
                        nc.tensor.matmul(pv[:, c0:], lhsT=v_sb[:, kt, h, :],
                                         rhs=pT[:, j, c0:],
                                         start=(n == 0),
                                         stop=(n == len(kts) - 1))
                        n += 1
                # normalize: attn^T[d, q] = attnU^T[d, q] / denom[q]
                # NOTE: reciprocal_approx_fast (custom DVE op) reading PSUM
                # directly returns garbage on HW -- copy to SBUF first.
                den = npool.tile([1, QB], F32, tag="den")
                nc.vector.tensor_copy(den, pv[DK:DK + 1, :])
                rec = npool.tile([1, QB], F32, tag="rec")
                nc.vector.reciprocal_approx_fast(rec, den)
                bc = npool.tile([64, QB], F32, tag="bc")
                nc.gpsimd.partition_broadcast(bc, rec)
                dst = attn_sb[pr:pr + DK, ft, qb * QB:(qb + 1) * QB]
                nc.vector.tensor_mul(dst, pv[0:DK, :], bc)

        def oproj(qb):
            # partial O-projection; bf16 staging per 128-token tile, DMA'd
            # out immediately so the final block's tail is short
            for qt in range(qb * QB // 128, (qb + 1) * QB // 128):
                ob = opool.tile([128, D], BF)
                for nch in range(D // NCH):
                    ps = psacc.tile([128, NCH], F32, tag="acc")
                    for hd in range(FT):
                        nc.tensor.matmul(
                            ps,
                            lhsT=attn_sb[:, hd, qt * 128:(qt + 1) * 128],
                            rhs=wo_sb[:, hd, nch * NCH:(nch + 1) * NCH],
                            start=(hd == 0), stop=(hd == FT - 1))
                    nc.vector.tensor_copy(ob[:, nch * NCH:(nch + 1) * NCH], ps)
                nc.sync.dma_start(out_r[qt], ob)

        # ---- software-pipelined main loop ----
        proj(0)
        for qb in range(NQB):
            attention(qb)
            if qb + 1 < NQB:
                proj(qb + 1)
            oproj(qb)

    nc.compile()
    return nc


def _get_program(mask2d: np.ndarray):
    key = hashlib.sha1(np.ascontiguousarray(mask2d).tobytes()).hexdigest()
    hit = _program_cache.get(key)
    if hit is not None:
        return hit
    plan, patterns = _classify_mask(mask2d)
    # combine the two masked tiles of a k-tile pair into one [128, 2, w]
    # pattern so a single strided DVE multiply covers both strips
    combos = {}
    for row in plan:
        for i in range(0, len(row) - 1, 2):
            (k0, m0, a0, b0), (k1, m1, a1, b1) = row[i], row[i + 1]
            if m0 is None or m1 is None:
                continue
            w = max(b0 - a0, b1 - a1)
            if a0 + w > QB or a1 + w > QB or w <= 0 or w > 128:
                continue
            key = (m0, m1, b0 - a0, b1 - a1)
            if key in combos:
                continue
            cp = np.ones((128, 2, w), BFNP)
            cp[:, 0, :b0 - a0] = patterns[m0]
            cp[:, 1, :b1 - a1] = patterns[m1]
            combos[key] = (len(patterns), w)
            patterns.append(cp.reshape(128, 2 * w))
    pw = max((p.shape[1] for p in patterns), default=0)
    nc = build_program(plan, len(patterns), pw, combos)
    if patterns:
        pat = np.zeros((len(patterns), 128, pw), BFNP)
        for i, p in enumerate(patterns):
            pat[i, :, :p.shape[1]] = p
        pat = np.ascontiguousarray(pat.transpose(1, 0, 2))  # (128, npat, pw)
    else:
        pat = None
    _program_cache[key] = (nc, pat)
    return nc, pat


def _pack_x(xT: np.ndarray) -> np.ndarray:
    # x^T (D, S) -> (128, NQB, DT, 512): [p, c, dt, j] = x^T[dt*128+p, c*512+j]
    return np.ascontiguousarray(
        xT.reshape(DT, 128, NQB, 512).transpose(1, 2, 0, 3))


def _pack_v(xT: np.ndarray) -> np.ndarray:
    # x^T (D, S) -> (128, NQB, 4, DT, 128): per-128-token-tile granularity
    return np.ascontiguousarray(
        xT.reshape(DT, 128, NQB, 4, 128).transpose(1, 2, 3, 0, 4))


def _pack_w(wT: np.ndarray) -> np.ndarray:
    # W^T slice (D, FPC) -> (128, DT, FPC)
    return np.ascontiguousarray(wT.reshape(DT, 128, FPC).transpose(1, 0, 2))


def kernel(**inputs) -> np.ndarray:
    global LAST_RESULT
    query = np.asarray(inputs["query"], np.float32)
    key = np.asarray(inputs["key"], np.float32)
    value = np.asarray(inputs["value"], np.float32)
    mask = np.asarray(inputs["mask"])
    Wq = np.asarray(inputs["Wq"], np.float32)
    bq = np.asarray(inputs["bq"], np.float32)
    Wk = np.asarray(inputs["Wk"], np.float32)
    bk = np.asarray(inputs["bk"], np.float32)
    Wv = np.asarray(inputs["Wv"], np.float32)
    bv = np.asarray(inputs["bv"], np.float32)
    Wo = np.asarray(inputs["Wo"], np.float32)
    bo = np.asarray(inputs["bo"], np.float32)

    nc, pat = _get_program(mask.reshape(S, S))

    WqT, WkT, WvT, WoT = Wq.T, Wk.T, Wv.T, Wo.T
    # packed per-batch inputs, shared across the 4 cores of a batch
    xp = {}
    for t, x in (("q", query), ("k", key)):
        xp[t] = [_pack_x(np.ascontiguousarray(x[b].T).astype(BFNP))
                 for b in range(B)]
    xp["v"] = [_pack_v(np.ascontiguousarray(value[b].T).astype(BFNP))
               for b in range(B)]
    xkq_b = [np.ascontiguousarray(
        np.stack([xp["k"][b], xp["q"][b]], axis=2)) for b in range(B)]
    in_maps = []
    for c in range(NCORE):
        b, g = divmod(c, GROUPS)
        f0 = g * FPC
        wall = np.stack([
            _pack_w(WkT[:, f0:f0 + FPC].astype(BFNP)),
            _pack_w(WqT[:, f0:f0 + FPC].astype(BFNP)),
            _pack_w(WvT[:, f0:f0 + FPC].astype(BFNP)),
        ], axis=1)  # (128, 3, DT, FPC)
        wop = np.ascontiguousarray(
            WoT[f0:f0 + FPC, :].astype(BFNP).reshape(FT, 128, D)
            .transpose(1, 0, 2))  # (128, FT, D)
        bqkp = np.ascontiguousarray(
            np.stack([bq[f0:f0 + FPC].reshape(FT, 128),
                      bk[f0:f0 + FPC].reshape(FT, 128)])
            .transpose(2, 0, 1).astype(np.float32))  # (128, 2, FT)
        m = {"xkq": xkq_b[b], "xv4": xp["v"][b], "wall": wall,
             "wo": wop, "bqk": bqkp}
        if pat is not None:
            m["masks"] = pat
        in_maps.append(m)

    res = run_bass_kernel_spmd(
        nc, in_maps, core_ids=list(range(NCORE)),
        trace=PROFILE,
        trace_cores=(TRACE_CORES if TRACE_CORES is not None
                     else (list(range(NCORE)) if PROFILE else None)),
    )
    LAST_RESULT = res

    host_bias = bo + bv @ WoT  # (D,) folded V/O biases, added once per batch
    out = np.empty((B, S, D), np.float32)
    for b in range(B):
        acc = res.results[b * GROUPS]["out"].astype(np.float32)
        for g in range(1, GROUPS):
            acc = acc + res.results[b * GROUPS + g]["out"].astype(np.float32)
        out[b] = acc + host_bias
    return out
